# revision 11
# baseline (speedup 1.0000x reference)
"""Trainium2 Bass kernel for nn_ClsTransformer (sparse kNN attention encoder).

Contract: kernel(**inputs) takes FULL unsharded inputs (x [8,1024,128] plus
stacked per-layer weights), shards batch across 8 NeuronCores (one batch
element per core, weights replicated), runs a fully fused per-core Bass/Tile
program, and returns the FULL [8,1024,128] output.
"""

import sys

sys.path.insert(0, "/opt/trn_rl_repo")

import numpy as np

import bass_rust
import concourse.bass as bass
import concourse.mybir as mybir
from concourse.tile import TileContext
from concourse.masks import make_identity
from concourse.bass_utils import run_bass_kernel_spmd


def legalize_waits(nc, max_waits=1):
    """This container's walrus rejects instructions carrying more than ~1
    semaphore wait ("Too many sync wait commands"). Hoist extra waits onto
    preceding single-wait NoOps on the same engine (same stall semantics:
    the engine's sequencer blocks in order)."""
    n_split = 0
    for f in nc.m.functions:
        for bb in f.blocks:
            out = []
            for inst in bb.instructions:
                si = inst.sync_info
                if si is not None and len(si.on_wait) > max_waits:
                    waits = list(si.on_wait)
                    for k, w in enumerate(waits[:-max_waits]):
                        nop = mybir.InstNoOp(name=f"{inst.name}-w{k}")
                        nop.engine = inst.engine
                        nop.sync_info = bass_rust.SyncInfo(on_wait=[w], on_update=[])
                        out.append(nop)
                        n_split += 1
                    si.on_wait = waits[-max_waits:]
                out.append(inst)
            bb.instructions[:] = out
    return n_split

# Problem constants (hardcoded per contract)
B, N, D = 8, 1024, 128
H, DK, DV = 8, 64, 64
MID = 2048
KNN = 30
L = 3
P = 128
NEG = -1.0e9
EPS = 1e-5
NCH = N // P          # 8 chunks of 128 rows
NC2 = N // 512        # 2 chunks of 512

F32 = mybir.dt.float32
F32R = mybir.dt.float32r
BF16 = mybir.dt.bfloat16
AF = mybir.ActivationFunctionType
OP = mybir.AluOpType

# f32r = full-rate fp32 matmul mode (1 cyc/row at free-dim>=256 vs 4 for f32).
# NOTE: f32r inputs must be produced already-rounded-to-f32r (walrus verifier);
# it is a reduced-precision format. Off until its numerics are measured.
FR_DIST = False  # distance matrix (selection-critical precision)
FR_PROJ = False  # QKV projections
FR_ATTN = False  # logits + AV
FR_MISC = False  # K=1/M=1 helper matmuls, WO, FFN
MASK_ON_GPSIMD = False  # build additive mask on GPSIMD instead of DVE


def _fr(ap, on):
    return ap.bitcast(F32R) if on else ap


def build_nc(legalize=True):
    nc = bass.Bass(trn_type="TRN2")
    x = nc.dram_tensor("x", [N, D], F32, kind="ExternalInput")
    wq = nc.dram_tensor("WQ", [L, D, H * DK], F32, kind="ExternalInput")
    wk = nc.dram_tensor("WK", [L, D, H * DK], F32, kind="ExternalInput")
    wv = nc.dram_tensor("WV", [L, D, H * DV], F32, kind="ExternalInput")
    wo = nc.dram_tensor("WO", [L, H * DV, D], F32, kind="ExternalInput")
    ff1 = nc.dram_tensor("FF1", [L, D, MID], F32, kind="ExternalInput")
    ff2 = nc.dram_tensor("FF2", [L, MID, D], F32, kind="ExternalInput")
    out = nc.dram_tensor("out", [N, D], F32, kind="ExternalOutput")

    with TileContext(nc) as tc:
        with (
            tc.tile_pool(name="const", bufs=1) as const,
            tc.tile_pool(name="persist", bufs=1) as persist,
            tc.tile_pool(name="hpool", bufs=1) as hpool,
            tc.tile_pool(name="wsmall", bufs=1) as wsmall,
            tc.tile_pool(name="wbig", bufs=1) as wbig,
            tc.tile_pool(name="work", bufs=2) as work,
            tc.tile_pool(name="rows", bufs=1) as rows,
            tc.tile_pool(name="scratch", bufs=1) as scratch,
            tc.tile_pool(name="ffp", bufs=3) as ffp,
            tc.tile_pool(name="psA", bufs=2, space="PSUM") as psA,   # [128,1024] = 2 banks
            tc.tile_pool(name="psB", bufs=3, space="PSUM") as psB,   # [*,512] = 1 bank
            tc.tile_pool(name="psT", bufs=1, space="PSUM") as psT,   # bf16 transpose staging
        ):
            # ---- constants ----
            ident_bf = const.tile([P, P], BF16)
            make_identity(nc, ident_bf)
            ident_f32 = const.tile([P, P], F32)
            make_identity(nc, ident_f32)
            neg_col = const.tile([P, 1], F32)
            nc.vector.memset(neg_col, -1.0)
            inv_col = const.tile([P, 1], F32)
            nc.vector.memset(inv_col, 1.0 / D)
            ones_row = const.tile([1, P], F32)
            nc.vector.memset(ones_row, 1.0)
            eps_row = const.tile([1, 1], F32)
            nc.vector.memset(eps_row, EPS)

            # ---- load x, transpose into hT [D, N] ----
            hT = hpool.tile([P, N], F32, tag="hT")
            for ch in range(NCH):
                xt = work.tile([P, P], F32, tag="xload")
                nc.sync.dma_start(xt[:], x[ch * P:(ch + 1) * P, :])
                pt = psB.tile([P, P], F32, tag="B")
                nc.tensor.transpose(pt[:], xt[:], ident_f32[:])
                nc.vector.tensor_copy(hT[:, ch * P:(ch + 1) * P], pt[:])

            mask_eng = nc.gpsimd if MASK_ON_GPSIMD else nc.vector

            for l in range(L):
                # ---- weights for this layer ----
                wq_t = wsmall.tile([P, H * DK], F32, tag="wq")
                wk_t = wsmall.tile([P, H * DK], F32, tag="wk")
                wv_t = wsmall.tile([P, H * DV], F32, tag="wv")
                wo_t = wsmall.tile([P, 4, P], F32, tag="wo")
                ff1_t = wbig.tile([P, MID], F32, tag="ff1")
                ff2_t = wbig.tile([P, MID // P, P], F32, tag="ff2")
                nc.sync.dma_start(wq_t[:], wq[l])
                nc.sync.dma_start(wk_t[:], wk[l])
                nc.sync.dma_start(wv_t[:], wv[l])
                nc.sync.dma_start(wo_t[:], wo[l].rearrange("(k p) d -> p k d", p=P))
                nc.sync.dma_start(ff1_t[:], ff1[l])
                nc.sync.dma_start(ff2_t[:], ff2[l].rearrange("(k p) d -> p k d", p=P))

                # ---- negsq[c] = -sum_d h[c,d]^2  (row [1, N]) ----
                hsq = scratch.tile([P, N], F32, tag="tmp32")
                nc.scalar.activation(hsq[:], hT[:], AF.Square)
                negsq = rows.tile([1, N], F32, tag="negsq")
                for c2 in range(NC2):
                    pns = psB.tile([1, 512], F32, tag="B")
                    nc.tensor.matmul(pns[:], _fr(neg_col[:], FR_MISC),
                                     _fr(hsq[:, c2 * 512:(c2 + 1) * 512], FR_MISC),
                                     start=True, stop=True)
                    nc.vector.tensor_copy(negsq[:, c2 * 512:(c2 + 1) * 512], pns[:])

                # hT2 = 2*hT (rhs of the distance matmul)
                hT2 = scratch.tile([P, N], F32, tag="hT2")
                nc.vector.tensor_scalar_mul(hT2[:], hT[:], 2.0)

                # ---- QKV projections ----
                # QT/KT: [hd, n] layout as [128, 4, N]; Q scaled by 1/sqrt(DK)
                QT = persist.tile([P, 4, N], F32, tag="QT")
                KT = persist.tile([P, 4, N], F32, tag="KT")
                for m in range(4):
                    for c2 in range(NC2):
                        pq = psB.tile([P, 512], F32, tag="B")
                        nc.tensor.matmul(pq[:], _fr(wq_t[:, m * P:(m + 1) * P], FR_PROJ),
                                         _fr(hT[:, c2 * 512:(c2 + 1) * 512], FR_PROJ),
                                         start=True, stop=True)
                        nc.scalar.mul(QT[:, m, c2 * 512:(c2 + 1) * 512], pq[:], 1.0 / np.sqrt(DK))
                        pk = psB.tile([P, 512], F32, tag="B")
                        nc.tensor.matmul(pk[:], _fr(wk_t[:, m * P:(m + 1) * P], FR_PROJ),
                                         _fr(hT[:, c2 * 512:(c2 + 1) * 512], FR_PROJ),
                                         start=True, stop=True)
                        nc.scalar.copy(KT[:, m, c2 * 512:(c2 + 1) * 512], pk[:])
                # V: [n, hd] layout with appended ones column: [128, NCH, H, DV+1]
                V = persist.tile([P, NCH, H, DV + 1], F32, tag="V")
                for ch in range(NCH):
                    pv = psB.tile([P, 512], F32, tag="B")
                    nc.tensor.matmul(pv[:], _fr(hT[:, ch * P:(ch + 1) * P], FR_PROJ),
                                     _fr(wv_t[:], FR_PROJ), start=True, stop=True)
                    nc.vector.tensor_copy(
                        V[:, ch, :, 0:DV],
                        pv[:].rearrange("p (h e) -> p h e", h=H))
                nc.vector.memset(V[:, :, :, DV:DV + 1], 1.0)

                # ---- distances + exact top-30 mask, transposed mask in bf16 ----
                maskT = persist.tile([P, NCH, N], BF16, tag="maskT")
                for ch in range(NCH):
                    pnd = psA.tile([P, N], F32, tag="A")
                    for c2 in range(NC2):
                        sl = slice(c2 * 512, (c2 + 1) * 512)
                        nc.tensor.matmul(pnd[:, sl], _fr(hT[:, ch * P:(ch + 1) * P], FR_DIST),
                                         _fr(hT2[:, sl], FR_DIST), start=True, stop=False)
                        nc.tensor.matmul(pnd[:, sl], _fr(ones_row[:], FR_MISC),
                                         _fr(negsq[:, sl], FR_MISC), start=False, stop=True)
                    nd = work.tile([P, N], F32, tag="nd")
                    nc.scalar.copy(nd[:], pnd[:])
                    wrk = work.tile([P, N], F32, tag="ndwork")
                    mx = rows.tile([P, 8], F32, tag="mx")
                    for rnd in range(4):
                        src = nd if rnd == 0 else wrk
                        nc.vector.max(mx[:], src[:])
                        if rnd == 3:
                            nc.vector.memset(mx[:, 6:8], NEG)
                        nc.vector.match_replace(out=wrk[:], in_to_replace=mx[:],
                                                in_values=src[:], imm_value=NEG)
                    # additive mask: selected -> 0, unselected -> -1e9 (bf16)
                    eq = work.tile([P, N], F32, tag="eq")
                    mask_eng.tensor_tensor(eq[:], nd[:], wrk[:], OP.is_equal)
                    madd = work.tile([P, N], BF16, tag="madd")
                    mask_eng.tensor_scalar_mul(madd[:], eq[:], NEG)
                    # transpose mask chunk [r=128, c=1024] -> maskT[c, :, r-chunk]
                    for g in range(2):
                        pt = psT.tile([P, 512], BF16, tag="T")
                        for j in range(4):
                            cs = g * 4 + j
                            nc.tensor.transpose(pt[:, j * P:(j + 1) * P],
                                                madd[:, cs * P:(cs + 1) * P], ident_bf[:])
                        nc.vector.tensor_copy(
                            maskT[:, g * 4:(g + 1) * 4, ch * P:(ch + 1) * P],
                            pt[:].rearrange("p (j r) -> p j r", j=4))

                # ---- attention (transposed orientation: [c, r]) ----
                oT = persist.tile([P, 4, N], F32, tag="oT")
                for h in range(H):
                    bp = (h % 2) * DK
                    qsl = QT[bp:bp + DK, h // 2, :]
                    ksl = KT[bp:bp + DK, h // 2, :]
                    po = [psB.tile([DV + 1, 512], F32, tag="B", name=f"po{h}_{i}") for i in range(NC2)]
                    for cs in range(NCH):
                        pl = psA.tile([P, N], F32, tag="A")
                        for rc in range(NC2):
                            sl = slice(rc * 512, (rc + 1) * 512)
                            nc.tensor.matmul(pl[:, sl], _fr(ksl[:, cs * P:(cs + 1) * P], FR_ATTN),
                                             _fr(qsl[:, sl], FR_ATTN), start=True, stop=False)
                            nc.tensor.matmul(pl[:, sl], ident_bf[:], maskT[:, cs, sl],
                                             start=False, stop=True)
                        eT = work.tile([P, N], F32, tag="eT")
                        nc.scalar.activation(eT[:], pl[:], AF.Exp)
                        for rc in range(NC2):
                            nc.tensor.matmul(po[rc][:], _fr(V[:, cs, h, :], FR_ATTN),
                                             _fr(eT[:, rc * 512:(rc + 1) * 512], FR_ATTN),
                                             start=(cs == 0), stop=(cs == NCH - 1),
                                             skip_group_check=True)
                    for rc in range(NC2):
                        rS = rows.tile([1, 512], F32, tag="rS")
                        nc.vector.reciprocal(rS[:], po[rc][DV:DV + 1, :])
                        prs = psB.tile([DV, 512], F32, tag="B")
                        nc.tensor.matmul(prs[:], _fr(ones_row[:, 0:DV], FR_MISC),
                                         _fr(rS[:], FR_MISC), start=True, stop=True)
                        rsb = work.tile([DV, 512], F32, tag="rsb")
                        nc.vector.tensor_copy(rsb[:], prs[:])
                        nc.vector.tensor_tensor(
                            oT[bp:bp + DV, h // 2, rc * 512:(rc + 1) * 512],
                            po[rc][0:DV, :], rsb[:], OP.mult)

                # ---- WO + residual + LN -> yT ----
                z1 = scratch.tile([P, N], F32, tag="z1")
                for rc in range(NC2):
                    sl = slice(rc * 512, (rc + 1) * 512)
                    pz = psB.tile([P, 512], F32, tag="B")
                    for kt in range(4):
                        nc.tensor.matmul(pz[:], _fr(wo_t[:, kt, :], FR_MISC),
                                         _fr(oT[:, kt, sl], FR_MISC),
                                         start=(kt == 0), stop=(kt == 3))
                    nc.vector.scalar_tensor_tensor(z1[:, sl], pz[:], 1.0, hT[:, sl],
                                                   OP.mult, OP.add)
                yT = scratch.tile([P, N], F32, tag="yT")
                layer_norm(nc, scratch, rows, psB, inv_col, ones_row, eps_row, z1, yT)

                # ---- FFN: relu(yT @ FF1) @ FF2 + residual + LN -> next hT ----
                z2 = scratch.tile([P, N], F32, tag="z2")
                for rc in range(NC2):
                    sl = slice(rc * 512, (rc + 1) * 512)
                    pz2 = psB.tile([P, 512], F32, tag="B")
                    for kt in range(MID // P):
                        pf = psB.tile([P, 512], F32, tag="B")
                        nc.tensor.matmul(pf[:], _fr(ff1_t[:, kt * P:(kt + 1) * P], FR_MISC),
                                         _fr(yT[:, sl], FR_MISC), start=True, stop=True)
                        ffs = ffp.tile([P, 512], F32, tag="ff")
                        nc.scalar.activation(ffs[:], pf[:], AF.Relu)
                        nc.tensor.matmul(pz2[:], _fr(ff2_t[:, kt, :], FR_MISC),
                                         _fr(ffs[:], FR_MISC),
                                         start=(kt == 0), stop=(kt == MID // P - 1),
                                         skip_group_check=True)
                    nc.vector.scalar_tensor_tensor(z2[:, sl], pz2[:], 1.0, yT[:, sl],
                                                   OP.mult, OP.add)
                hT = hpool.tile([P, N], F32, tag="hT")
                layer_norm(nc, scratch, rows, psB, inv_col, ones_row, eps_row, z2, hT)

            # ---- output: transpose hT back to [N, D] ----
            for ch in range(NCH):
                pt = psB.tile([P, P], F32, tag="B")
                nc.tensor.transpose(pt[:], hT[:, ch * P:(ch + 1) * P], ident_f32[:])
                ot = work.tile([P, P], F32, tag="xload")
                nc.vector.tensor_copy(ot[:], pt[:])
                nc.sync.dma_start(out[ch * P:(ch + 1) * P, :], ot[:])
    if legalize:
        legalize_waits(nc)
    return nc


def layer_norm(nc, scratch, rows, psB, inv_col, ones_row, eps_row, zT, outT):
    """outT = (zT - mean) * rsqrt(var + eps), stats over the partition (D) axis.

    mean/E[z^2] via ones-matmuls, rstd = exp(-0.5*log(var+eps)) on ACT
    (the Sqrt table is too coarse), broadcast back via K=1 outer-product MMs.
    """
    zsq = scratch.tile([P, N], F32, tag="tmp32")
    nc.scalar.activation(zsq[:], zT[:], AF.Square)
    mean = rows.tile([1, N], F32, tag="mean")
    msq = rows.tile([1, N], F32, tag="msq")
    for c2 in range(NC2):
        sl = slice(c2 * 512, (c2 + 1) * 512)
        pm = psB.tile([1, 512], F32, tag="B")
        nc.tensor.matmul(pm[:], _fr(inv_col[:], FR_MISC), _fr(zT[:, sl], FR_MISC),
                         start=True, stop=True)
        nc.vector.tensor_copy(mean[:, sl], pm[:])
        pm2 = psB.tile([1, 512], F32, tag="B")
        nc.tensor.matmul(pm2[:], _fr(inv_col[:], FR_MISC), _fr(zsq[:, sl], FR_MISC),
                         start=True, stop=True)
        nc.vector.tensor_copy(msq[:, sl], pm2[:])
    rowtmp = rows.tile([1, N], F32, tag="rowtmp")
    nc.vector.tensor_tensor(rowtmp[:], mean[:], mean[:], OP.mult)
    nc.vector.tensor_tensor(msq[:], msq[:], rowtmp[:], OP.subtract)   # msq := var
    nc.scalar.activation(rowtmp[:], msq[:], AF.Ln, bias=eps_row[:])   # rowtmp := ln(var+eps)
    nc.scalar.activation(msq[:], rowtmp[:], AF.Exp, scale=-0.5)       # msq := rstd
    rstd = msq
    brow = rowtmp
    nc.vector.scalar_tensor_tensor(brow[:], mean[:], -1.0, rstd[:], OP.mult, OP.mult)
    for c2 in range(NC2):
        sl = slice(c2 * 512, (c2 + 1) * 512)
        pa = psB.tile([P, 512], F32, tag="B")
        nc.tensor.matmul(pa[:], _fr(ones_row[:], FR_MISC), _fr(rstd[:, sl], FR_MISC),
                         start=True, stop=True)
        pb = psB.tile([P, 512], F32, tag="B")
        nc.tensor.matmul(pb[:], _fr(ones_row[:], FR_MISC), _fr(brow[:, sl], FR_MISC),
                         start=True, stop=True)
        nc.vector.tensor_tensor(outT[:, sl], zT[:, sl], pa[:], OP.mult)
        nc.vector.tensor_tensor(outT[:, sl], outT[:, sl], pb[:], OP.add)


_nc_cache = None


def kernel(**inputs):
    global _nc_cache
    if _nc_cache is None:
        _nc_cache = build_nc()
    nc = _nc_cache
    x = np.ascontiguousarray(inputs["x"], dtype=np.float32)
    shared = {k: np.ascontiguousarray(np.asarray(inputs[k]), dtype=np.float32)
              for k in ("WQ", "WK", "WV", "WO", "FF1", "FF2")}
    in_maps = [dict(x=x[b], **shared) for b in range(B)]
    res = run_bass_kernel_spmd(nc, in_maps, core_ids=list(range(B)))
    return np.stack([res.results[b]["out"] for b in range(B)], axis=0)


if __name__ == "__main__":
    nc = build_nc()
    print("built ok")


# revision 13
# speedup vs baseline: 62.6684x; 62.6684x over previous
"""Trainium2 Bass kernel for nn_ClsTransformer (sparse kNN attention encoder).

Contract: kernel(**inputs) takes FULL unsharded inputs (x [8,1024,128] plus
stacked per-layer weights), shards batch across 8 NeuronCores (one batch
element per core, weights replicated), runs a fully fused per-core Bass/Tile
program, and returns the FULL [8,1024,128] output.
"""

import sys

sys.path.insert(0, "/opt/trn_rl_repo")

import numpy as np

import bass_rust
import concourse.bass as bass
import concourse.mybir as mybir
from concourse.tile import TileContext
from concourse.masks import make_identity
from concourse.bass_utils import run_bass_kernel_spmd


def legalize_waits(nc, max_waits=1):
    """This container's walrus rejects instructions carrying more than ~1
    semaphore wait ("Too many sync wait commands"). Hoist extra waits onto
    preceding single-wait NoOps on the same engine (same stall semantics:
    the engine's sequencer blocks in order)."""
    n_split = 0
    for f in nc.m.functions:
        for bb in f.blocks:
            out = []
            for inst in bb.instructions:
                si = inst.sync_info
                if si is not None and len(si.on_wait) > max_waits:
                    waits = list(si.on_wait)
                    for k, w in enumerate(waits[:-max_waits]):
                        nop = mybir.InstNoOp(name=f"{inst.name}-w{k}")
                        nop.engine = inst.engine
                        nop.sync_info = bass_rust.SyncInfo(on_wait=[w], on_update=[])
                        out.append(nop)
                        n_split += 1
                    si.on_wait = waits[-max_waits:]
                out.append(inst)
            bb.instructions[:] = out
    return n_split

# Problem constants (hardcoded per contract)
B, N, D = 8, 1024, 128
H, DK, DV = 8, 64, 64
MID = 2048
KNN = 30
L = 3
P = 128
NEG = -1.0e9
EPS = 1e-5
NCH = N // P          # 8 chunks of 128 rows
NC2 = N // 512        # 2 chunks of 512

F32 = mybir.dt.float32
F32R = mybir.dt.float32r
BF16 = mybir.dt.bfloat16
AF = mybir.ActivationFunctionType
OP = mybir.AluOpType

# f32r = full-rate fp32 matmul mode (1 cyc/row at free-dim>=256 vs 4 for f32).
# NOTE: f32r inputs must be produced already-rounded-to-f32r (walrus verifier);
# it is a reduced-precision format. Off until its numerics are measured.
FR_DIST = False  # distance matrix (selection-critical precision)
FR_PROJ = False  # QKV projections
FR_ATTN = True   # logits + AV via f32r tiles (QT/KT/eT/V)
FR_MISC = False  # K=1/M=1 helper matmuls, WO, FFN
MASK_ON_GPSIMD = False  # build additive mask on GPSIMD instead of DVE


def _fr(ap, on):
    return ap.bitcast(F32R) if on else ap


def build_nc(legalize=True):
    nc = bass.Bass(trn_type="TRN2")
    x = nc.dram_tensor("x", [N, D], F32, kind="ExternalInput")
    wq = nc.dram_tensor("WQ", [L, D, H * DK], F32, kind="ExternalInput")
    wk = nc.dram_tensor("WK", [L, D, H * DK], F32, kind="ExternalInput")
    wv = nc.dram_tensor("WV", [L, D, H * DV], F32, kind="ExternalInput")
    wo = nc.dram_tensor("WO", [L, H * DV, D], F32, kind="ExternalInput")
    ff1 = nc.dram_tensor("FF1", [L, D, MID], F32, kind="ExternalInput")
    ff2 = nc.dram_tensor("FF2", [L, MID, D], F32, kind="ExternalInput")
    out = nc.dram_tensor("out", [N, D], F32, kind="ExternalOutput")

    with TileContext(nc) as tc:
        with (
            tc.tile_pool(name="const", bufs=1) as const,
            tc.tile_pool(name="persist", bufs=1) as persist,
            tc.tile_pool(name="hpool", bufs=1) as hpool,
            tc.tile_pool(name="wsmall", bufs=1) as wsmall,
            tc.tile_pool(name="wbig", bufs=1) as wbig,
            tc.tile_pool(name="work", bufs=2) as work,
            tc.tile_pool(name="rows", bufs=1) as rows,
            tc.tile_pool(name="scratch", bufs=1) as scratch,
            tc.tile_pool(name="ffp", bufs=3) as ffp,
            tc.tile_pool(name="psA", bufs=2, space="PSUM") as psA,   # [128,1024] = 2 banks
            tc.tile_pool(name="psB", bufs=3, space="PSUM") as psB,   # [*,512] = 1 bank
            tc.tile_pool(name="psT", bufs=1, space="PSUM") as psT,   # bf16 transpose staging
        ):
            # ---- constants ----
            ident_bf = const.tile([P, P], BF16)
            make_identity(nc, ident_bf)
            ident_f32 = const.tile([P, P], F32)
            make_identity(nc, ident_f32)
            neg_col = const.tile([P, 1], F32)
            nc.vector.memset(neg_col, -1.0)
            inv_col = const.tile([P, 1], F32)
            nc.vector.memset(inv_col, 1.0 / D)
            ones_row = const.tile([1, P], F32)
            nc.vector.memset(ones_row, 1.0)
            eps_row = const.tile([1, 1], F32)
            nc.vector.memset(eps_row, EPS)
            ones_col128 = const.tile([P, 1], F32)
            nc.vector.memset(ones_col128, 1.0)

            # ---- load x, transpose into hT [D, N] ----
            hT = hpool.tile([P, N], F32, tag="hT")
            for ch in range(NCH):
                xt = work.tile([P, P], F32, tag="xload")
                nc.sync.dma_start(xt[:], x[ch * P:(ch + 1) * P, :])
                pt = psB.tile([P, P], F32, tag="B")
                nc.tensor.transpose(pt[:], xt[:], ident_f32[:])
                nc.vector.tensor_copy(hT[:, ch * P:(ch + 1) * P], pt[:])

            mask_eng = nc.gpsimd if MASK_ON_GPSIMD else nc.vector

            for l in range(L):
                # ---- weights for this layer ----
                wq_t = wsmall.tile([P, H * DK], F32, tag="wq")
                wk_t = wsmall.tile([P, H * DK], F32, tag="wk")
                wv_t = wsmall.tile([P, H * DV], F32, tag="wv")
                wo_t = wsmall.tile([P, 4, P], F32, tag="wo")
                ff1_t = wbig.tile([P, MID], F32, tag="ff1")
                ff2_t = wbig.tile([P, MID // P, P], F32, tag="ff2")
                nc.sync.dma_start(wq_t[:], wq[l])
                nc.sync.dma_start(wk_t[:], wk[l])
                nc.sync.dma_start(wv_t[:], wv[l])
                nc.sync.dma_start(wo_t[:], wo[l].rearrange("(k p) d -> p k d", p=P))
                nc.sync.dma_start(ff1_t[:], ff1[l])
                nc.sync.dma_start(ff2_t[:], ff2[l].rearrange("(k p) d -> p k d", p=P))

                # ---- negsq[c] = -sum_d h[c,d]^2  (row [1, N]) ----
                hsq = scratch.tile([P, N], F32, tag="tmp32")
                nc.scalar.activation(hsq[:], hT[:], AF.Square)
                negsq = rows.tile([1, N], F32, tag="negsq")
                for c2 in range(NC2):
                    pns = psB.tile([1, 512], F32, tag="B")
                    nc.tensor.matmul(pns[:], _fr(neg_col[:], FR_MISC),
                                     _fr(hsq[:, c2 * 512:(c2 + 1) * 512], FR_MISC),
                                     start=True, stop=True)
                    nc.vector.tensor_copy(negsq[:, c2 * 512:(c2 + 1) * 512], pns[:])

                # hT2 = 2*hT (rhs of the distance matmul)
                hT2 = scratch.tile([P, N], F32, tag="hT2")
                nc.vector.tensor_scalar_mul(hT2[:], hT[:], 2.0)

                # ---- QKV projections ----
                # QT/KT: [hd, n] layout as [128, 4, N]; Q scaled by 1/sqrt(DK)
                QT = persist.tile([P, 4, N], F32R if FR_ATTN else F32, tag="QT")
                KT = persist.tile([P, 4, N], F32R if FR_ATTN else F32, tag="KT")
                for m in range(4):
                    for c2 in range(NC2):
                        pq = psB.tile([P, 512], F32, tag="B")
                        nc.tensor.matmul(pq[:], _fr(wq_t[:, m * P:(m + 1) * P], FR_PROJ),
                                         _fr(hT[:, c2 * 512:(c2 + 1) * 512], FR_PROJ),
                                         start=True, stop=True)
                        nc.scalar.mul(QT[:, m, c2 * 512:(c2 + 1) * 512], pq[:], 1.0 / np.sqrt(DK))
                        pk = psB.tile([P, 512], F32, tag="B")
                        nc.tensor.matmul(pk[:], _fr(wk_t[:, m * P:(m + 1) * P], FR_PROJ),
                                         _fr(hT[:, c2 * 512:(c2 + 1) * 512], FR_PROJ),
                                         start=True, stop=True)
                        nc.scalar.copy(KT[:, m, c2 * 512:(c2 + 1) * 512], pk[:])
                # V: [n, hd] layout with appended ones column: [128, NCH, H, DV+1]
                V = persist.tile([P, NCH, H, DV + 1], F32R if FR_ATTN else F32, tag="V")
                for ch in range(NCH):
                    pv = psB.tile([P, 512], F32, tag="B")
                    nc.tensor.matmul(pv[:], _fr(hT[:, ch * P:(ch + 1) * P], FR_PROJ),
                                     _fr(wv_t[:], FR_PROJ), start=True, stop=True)
                    nc.vector.tensor_copy(
                        V[:, ch, :, 0:DV],
                        pv[:].rearrange("p (h e) -> p h e", h=H))
                for _ch in range(NCH):
                    nc.vector.tensor_copy(V[:, _ch, :, DV:DV + 1],
                                          ones_col128.to_broadcast([P, H, 1]))

                # ---- distances + exact top-30 mask, transposed mask in bf16 ----
                maskT = persist.tile([P, NCH, N], BF16, tag="maskT")
                for ch in range(NCH):
                    pnd = psA.tile([P, N], F32, tag="A")
                    for c2 in range(NC2):
                        sl = slice(c2 * 512, (c2 + 1) * 512)
                        nc.tensor.matmul(pnd[:, sl], _fr(hT[:, ch * P:(ch + 1) * P], FR_DIST),
                                         _fr(hT2[:, sl], FR_DIST), start=True, stop=False)
                        nc.tensor.matmul(pnd[:, sl], _fr(ones_row[:], FR_MISC),
                                         _fr(negsq[:, sl], FR_MISC), start=False, stop=True)
                    nd = work.tile([P, N], F32, tag="nd")
                    nc.scalar.copy(nd[:], pnd[:])
                    wrk = work.tile([P, N], F32, tag="ndwork")
                    mx = rows.tile([P, 8], F32, tag="mx")
                    for rnd in range(4):
                        src = nd if rnd == 0 else wrk
                        nc.vector.max(mx[:], src[:])
                        if rnd == 3:
                            nc.vector.memset(mx[:, 6:8], NEG)
                        nc.vector.match_replace(out=wrk[:], in_to_replace=mx[:],
                                                in_values=src[:], imm_value=NEG)
                    # additive mask: selected -> 0, unselected -> -1e9 (bf16)
                    eq = work.tile([P, N], F32, tag="eq")
                    mask_eng.tensor_tensor(eq[:], nd[:], wrk[:], OP.is_equal)
                    madd = work.tile([P, N], BF16, tag="madd")
                    mask_eng.tensor_scalar_mul(madd[:], eq[:], NEG)
                    # transpose mask chunk [r=128, c=1024] -> maskT[c, :, r-chunk]
                    for g in range(2):
                        pt = psT.tile([P, 512], BF16, tag="T")
                        for j in range(4):
                            cs = g * 4 + j
                            nc.tensor.transpose(pt[:, j * P:(j + 1) * P],
                                                madd[:, cs * P:(cs + 1) * P], ident_bf[:])
                        nc.vector.tensor_copy(
                            maskT[:, g * 4:(g + 1) * 4, ch * P:(ch + 1) * P],
                            pt[:].rearrange("p (j r) -> p j r", j=4))

                # ---- attention (transposed orientation: [c, r]) ----
                oT = persist.tile([P, 4, N], F32, tag="oT")
                for h in range(H):
                    bp = (h % 2) * DK
                    qsl = QT[bp:bp + DK, h // 2, :]
                    ksl = KT[bp:bp + DK, h // 2, :]
                    po = [psB.tile([DV + 1, 512], F32, tag="B", name=f"po{h}_{i}") for i in range(NC2)]
                    for cs in range(NCH):
                        pl = psA.tile([P, N], F32, tag="A")
                        for rc in range(NC2):
                            sl = slice(rc * 512, (rc + 1) * 512)
                            nc.tensor.matmul(pl[:, sl], ksl[:, cs * P:(cs + 1) * P],
                                             qsl[:, sl], start=True, stop=False)
                            nc.tensor.matmul(pl[:, sl], ident_bf[:], maskT[:, cs, sl],
                                             start=False, stop=True)
                        eT = work.tile([P, N], F32R if FR_ATTN else F32, tag="eT")
                        nc.scalar.activation(eT[:], pl[:], AF.Exp)
                        for rc in range(NC2):
                            nc.tensor.matmul(po[rc][:], V[:, cs, h, :],
                                             eT[:, rc * 512:(rc + 1) * 512],
                                             start=(cs == 0), stop=(cs == NCH - 1),
                                             skip_group_check=True)
                    for rc in range(NC2):
                        rS = rows.tile([1, 512], F32, tag="rS")
                        nc.vector.reciprocal(rS[:], po[rc][DV:DV + 1, :])
                        prs = psB.tile([DV, 512], F32, tag="B")
                        nc.tensor.matmul(prs[:], _fr(ones_row[:, 0:DV], FR_MISC),
                                         _fr(rS[:], FR_MISC), start=True, stop=True)
                        rsb = work.tile([DV, 512], F32, tag="rsb")
                        nc.vector.tensor_copy(rsb[:], prs[:])
                        nc.vector.tensor_tensor(
                            oT[bp:bp + DV, h // 2, rc * 512:(rc + 1) * 512],
                            po[rc][0:DV, :], rsb[:], OP.mult)

                # ---- WO + residual + LN -> yT ----
                z1 = scratch.tile([P, N], F32, tag="z1")
                for rc in range(NC2):
                    sl = slice(rc * 512, (rc + 1) * 512)
                    pz = psB.tile([P, 512], F32, tag="B")
                    for kt in range(4):
                        nc.tensor.matmul(pz[:], _fr(wo_t[:, kt, :], FR_MISC),
                                         _fr(oT[:, kt, sl], FR_MISC),
                                         start=(kt == 0), stop=(kt == 3))
                    nc.vector.scalar_tensor_tensor(z1[:, sl], pz[:], 1.0, hT[:, sl],
                                                   OP.mult, OP.add)
                yT = scratch.tile([P, N], F32, tag="yT")
                layer_norm(nc, scratch, rows, psB, inv_col, ones_row, eps_row, z1, yT)

                # ---- FFN: relu(yT @ FF1) @ FF2 + residual + LN -> next hT ----
                z2 = scratch.tile([P, N], F32, tag="z2")
                for rc in range(NC2):
                    sl = slice(rc * 512, (rc + 1) * 512)
                    pz2 = psB.tile([P, 512], F32, tag="B")
                    for kt in range(MID // P):
                        pf = psB.tile([P, 512], F32, tag="B")
                        nc.tensor.matmul(pf[:], _fr(ff1_t[:, kt * P:(kt + 1) * P], FR_MISC),
                                         _fr(yT[:, sl], FR_MISC), start=True, stop=True)
                        ffs = ffp.tile([P, 512], F32, tag="ff")
                        nc.scalar.activation(ffs[:], pf[:], AF.Relu)
                        nc.tensor.matmul(pz2[:], _fr(ff2_t[:, kt, :], FR_MISC),
                                         _fr(ffs[:], FR_MISC),
                                         start=(kt == 0), stop=(kt == MID // P - 1),
                                         skip_group_check=True)
                    nc.vector.scalar_tensor_tensor(z2[:, sl], pz2[:], 1.0, yT[:, sl],
                                                   OP.mult, OP.add)
                hT = hpool.tile([P, N], F32, tag="hT")
                layer_norm(nc, scratch, rows, psB, inv_col, ones_row, eps_row, z2, hT)

            # ---- output: transpose hT back to [N, D] ----
            for ch in range(NCH):
                pt = psB.tile([P, P], F32, tag="B")
                nc.tensor.transpose(pt[:], hT[:, ch * P:(ch + 1) * P], ident_f32[:])
                ot = work.tile([P, P], F32, tag="xload")
                nc.vector.tensor_copy(ot[:], pt[:])
                nc.sync.dma_start(out[ch * P:(ch + 1) * P, :], ot[:])
    if legalize:
        legalize_waits(nc)
    return nc


def layer_norm(nc, scratch, rows, psB, inv_col, ones_row, eps_row, zT, outT):
    """outT = (zT - mean) * rsqrt(var + eps), stats over the partition (D) axis.

    mean/E[z^2] via ones-matmuls, rstd = exp(-0.5*log(var+eps)) on ACT
    (the Sqrt table is too coarse), broadcast back via K=1 outer-product MMs.
    """
    zsq = scratch.tile([P, N], F32, tag="tmp32")
    nc.scalar.activation(zsq[:], zT[:], AF.Square)
    mean = rows.tile([1, N], F32, tag="mean")
    msq = rows.tile([1, N], F32, tag="msq")
    for c2 in range(NC2):
        sl = slice(c2 * 512, (c2 + 1) * 512)
        pm = psB.tile([1, 512], F32, tag="B")
        nc.tensor.matmul(pm[:], _fr(inv_col[:], FR_MISC), _fr(zT[:, sl], FR_MISC),
                         start=True, stop=True)
        nc.vector.tensor_copy(mean[:, sl], pm[:])
        pm2 = psB.tile([1, 512], F32, tag="B")
        nc.tensor.matmul(pm2[:], _fr(inv_col[:], FR_MISC), _fr(zsq[:, sl], FR_MISC),
                         start=True, stop=True)
        nc.vector.tensor_copy(msq[:, sl], pm2[:])
    rowtmp = rows.tile([1, N], F32, tag="rowtmp")
    nc.vector.tensor_tensor(rowtmp[:], mean[:], mean[:], OP.mult)
    nc.vector.tensor_tensor(msq[:], msq[:], rowtmp[:], OP.subtract)   # msq := var
    nc.scalar.activation(rowtmp[:], msq[:], AF.Ln, bias=eps_row[:])   # rowtmp := ln(var+eps)
    nc.scalar.activation(msq[:], rowtmp[:], AF.Exp, scale=-0.5)       # msq := rstd
    rstd = msq
    brow = rowtmp
    nc.vector.scalar_tensor_tensor(brow[:], mean[:], -1.0, rstd[:], OP.mult, OP.mult)
    for c2 in range(NC2):
        sl = slice(c2 * 512, (c2 + 1) * 512)
        pa = psB.tile([P, 512], F32, tag="B")
        nc.tensor.matmul(pa[:], _fr(ones_row[:], FR_MISC), _fr(rstd[:, sl], FR_MISC),
                         start=True, stop=True)
        pb = psB.tile([P, 512], F32, tag="B")
        nc.tensor.matmul(pb[:], _fr(ones_row[:], FR_MISC), _fr(brow[:, sl], FR_MISC),
                         start=True, stop=True)
        nc.vector.tensor_tensor(outT[:, sl], zT[:, sl], pa[:], OP.mult)
        nc.vector.tensor_tensor(outT[:, sl], outT[:, sl], pb[:], OP.add)


_nc_cache = None


def kernel(**inputs):
    global _nc_cache
    if _nc_cache is None:
        _nc_cache = build_nc()
    nc = _nc_cache
    x = np.ascontiguousarray(inputs["x"], dtype=np.float32)
    shared = {k: np.ascontiguousarray(np.asarray(inputs[k]), dtype=np.float32)
              for k in ("WQ", "WK", "WV", "WO", "FF1", "FF2")}
    in_maps = [dict(x=x[b], **shared) for b in range(B)]
    res = run_bass_kernel_spmd(nc, in_maps, core_ids=list(range(B)))
    return np.stack([res.results[b]["out"] for b in range(B)], axis=0)


if __name__ == "__main__":
    nc = build_nc()
    print("built ok")
